# revision 1
# baseline (speedup 1.0000x reference)
"""DenseGCNBlock on 8 trn2 NeuronCores (Bass/Tile) — dense-adjacency version,
chunk-major software pipelining across layers.

Math (per layer l, weight W_l [C_l+16, 128]):
    msg_e = concat(cat[src_e], ea_e) @ W_l + b_l
    z_l   = segment_sum(msg, dst) / max(counts, 1)
Linearity splits z_l into blocks:
    z_l = (sum_m agg_m @ Wx_block_m + [EA|counts] @ [We_l;b_l]) / denom

Aggregation is a dense matmul against the 0/1 adjacency: aggT[ch, dst] =
sum_s H_s^T @ AT_s, AT [10240 src, 1280 dst] (padded ids) host-built, fp8e3
(exact small ints), RESIDENT in SBUF.  H is the full node-feature matrix.

Pipelining: node tiles are permuted: A half = windows 0-3 of every core
(32 tiles), B half = windows 4-9 (48 tiles).  Per product the PE order is
  [A-srcs x chunks 0,1,2] [B-srcs chunk0] z w0-3 -> AllGather-A
  [B-srcs chunk1] z w4-7 [B-srcs chunk2] z w8-9 -> AllGather-B
so the next product's A-block (25 us) + B chunk0 (10 us) hide both
AllGathers and H reloads.

EA/counts (product 0 only): per edge tile, lhsT=P_t (one-hot dst), rhs=ea_t
(N=17) accumulating [dst, 17] per window; counts = col 16; PE-transpose to
eaT [17, dst] for the z matmuls.
"""
import os
import sys

sys.path.insert(0, "/opt/trn_rl_repo")

import numpy as np
import ml_dtypes

_NPROD = int(os.environ.get("KERNEL_NPROD", "4"))  # debug knob: products to run
_NCC = int(os.environ.get("KERNEL_NCC", "3"))      # debug knob: collectives to run
_ATDT = os.environ.get("KERNEL_ATDT", "fp8")       # adjacency dtype: fp8 | fp16

N_NODES = 10000
N_EDGES = 320000
CH = 128
EDGE_DIM = 16
EAD = EDGE_DIM + 1  # 16 attrs + ones column (counts)
N_CORES = 8
NPC = N_NODES // N_CORES   # 1250 real nodes per core
WPC = 10                   # windows per core
WA = 4                     # windows in the A half
NPP = WPC * 128            # 1280 padded nodes per core
NPAD = N_CORES * NPP       # 10240 padded nodes total
ST = NPAD // 128           # 80 src tiles
STA = N_CORES * WA         # 32 src tiles in the A half
NHA = STA * 128            # 4096 nodes in the A half
NHB = NPAD - NHA           # 6144 nodes in the B half
CHUNKS = ((0, 512), (512, 1024), (1024, NPP))
# windows covered by each chunk (128 dst per window)
CHUNK_WINS = ((0, 1, 2, 3), (4, 5, 6, 7), (8, 9))
PAD_OFF = np.float16(255.0)

# node-tile permutation: A half = (c, w<WA), B half = (c, w>=WA)
PERM = [10 * c + w for c in range(N_CORES) for w in range(WA)] + \
       [10 * c + w for c in range(N_CORES) for w in range(WA, WPC)]


def _prep(edge_index, edge_attr, at_np_dt):
    """Sort edges by dst; build per-core dense (padded, tile-permuted)
    adjacency AT, one-hot dst tiles P, packed edge attrs."""
    src = np.asarray(edge_index[0], dtype=np.int64)
    dst = np.asarray(edge_index[1], dtype=np.int64)
    ea = np.asarray(edge_attr, dtype=np.float32)

    order = np.argsort(dst, kind="stable")
    src_s = src[order]
    dst_s = dst[order]
    ea_s = ea[order]
    spad_s = (src_s // NPC) * NPP + (src_s % NPC)  # padded global src ids

    bounds = []
    for c in range(N_CORES):
        base = NPC * c
        for w in range(WPC):
            lo = base + 128 * w
            hi = min(base + 128 * (w + 1), base + NPC)
            bounds.append((lo, hi))
    starts = np.searchsorted(dst_s, [b[0] for b in bounds], side="left")
    ends = np.searchsorted(dst_s, [b[1] for b in bounds], side="left")
    counts_w = ends - starts
    T = max(1, int(np.max((counts_w + 127) // 128)))
    EPW = T * 128
    NT = WPC * T

    offs_all = np.full((N_CORES, WPC * EPW), PAD_OFF, np.float16)
    ea_all = np.zeros((N_CORES, WPC * EPW, EAD), np.float16)
    for bi, (lo, hi) in enumerate(bounds):
        c, w = divmod(bi, WPC)
        s, e = starts[bi], ends[bi]
        n = e - s
        o = w * EPW
        offs_all[c, o : o + n] = (dst_s[s:e] - lo).astype(np.float16)
        ea_all[c, o : o + n, :EDGE_DIM] = ea_s[s:e].astype(np.float16)
        ea_all[c, o : o + n, EDGE_DIM] = 1.0

    at = np.zeros((N_CORES, 128, ST, NPP), at_np_dt)
    iota128 = np.arange(128, dtype=np.float16)
    p_pk = np.zeros((N_CORES, 128, NT * 128), ml_dtypes.float8_e3m4)
    ea_pk = np.zeros((N_CORES, 128, NT, EAD), np.float16)
    for c in range(N_CORES):
        s, e = starts[WPC * c], ends[WPC * c + WPC - 1]
        flat = spad_s[s:e] * NPP + (dst_s[s:e] - NPC * c)
        cnt = np.bincount(flat, minlength=NPAD * NPP)
        atc = cnt.astype(np.float32).reshape(ST, 128, NPP)[PERM]
        at[c] = atc.transpose(1, 0, 2).astype(at_np_dt)
        oh = offs_all[c].reshape(NT, 128)[:, :, None] == iota128[None, None, :]
        p_pk[c] = oh.transpose(1, 0, 2).reshape(128, NT * 128).astype(ml_dtypes.float8_e3m4)
        ea_pk[c] = ea_all[c].reshape(NT, 128, EAD).transpose(1, 0, 2)
    return at, p_pk, ea_pk, T


def _build(T, use_fp8, mybir, bass, tile, bacc):
    """Build the SPMD Bass program (same for all cores)."""
    fp16 = mybir.dt.float16
    f32 = mybir.dt.float32
    at_dt = mybir.dt.float8e3 if use_fp8 else fp16
    NT = WPC * T

    nc = bacc.Bacc("TRN2", num_devices=N_CORES)
    # x16 already in permuted-tile layout
    x16 = nc.dram_tensor("x16", [128, ST * CH], fp16, kind="ExternalInput")
    at_d = nc.dram_tensor("at", [128, ST, NPP], at_dt, kind="ExternalInput")
    p_d = nc.dram_tensor("pt", [128, NT * 128], mybir.dt.float8e3, kind="ExternalInput")
    ea_d = nc.dram_tensor("ea", [128, NT * EAD], fp16, kind="ExternalInput")
    wx_d = nc.dram_tensor("wx", [7, 128, 128], fp16, kind="ExternalInput")
    wep_d = nc.dram_tensor("wep", [4, EAD, 128], fp16, kind="ExternalInput")
    id_d = nc.dram_tensor("id128", [128, 128], fp16, kind="ExternalInput")
    out_d = nc.dram_tensor("zout", [NPC, CH], f32, kind="ExternalOutput")

    PIECES = {0: [(0, 0)], 1: [(1, 1)], 2: [(1, 2), (2, 3)], 3: [(1, 4), (2, 5), (3, 6)]}
    wsizes = [128] * (WPC - 1) + [NPC - 128 * (WPC - 1)]

    with tile.TileContext(nc) as tc:
        with tc.tile_pool(name="singles", bufs=1) as singles, \
             tc.tile_pool(name="ppool", bufs=4) as ppool, \
             tc.tile_pool(name="zpool", bufs=4) as zpool, \
             tc.tile_pool(name="small", bufs=2) as small, \
             tc.tile_pool(name="ps_agg", bufs=1, space="PSUM") as ps_agg, \
             tc.tile_pool(name="ps_ea", bufs=1, space="PSUM") as ps_ea, \
             tc.tile_pool(name="ps_z", bufs=3, space="PSUM") as ps_z, \
             tc.tile_pool(name="dram", bufs=1, space="DRAM") as dram:

            id_t = singles.tile([128, 128], fp16)
            nc.sync.dma_start(out=id_t[:, :], in_=id_d[:, :])
            wx_t = singles.tile([128, 7, 128], fp16)
            nc.sync.dma_start(out=wx_t[:, :, :], in_=wx_d[:, :, :].rearrange("k p j -> p k j"))
            wep_t = singles.tile([EAD, 4, 128], fp16)
            nc.sync.dma_start(out=wep_t[:, :, :], in_=wep_d[:, :, :].rearrange("l p j -> p l j"))
            ea_t = singles.tile([128, NT, EAD], fp16)
            nc.scalar.dma_start(out=ea_t[:, :2 * T, :],
                                in_=ea_d[:, :2 * T * EAD].rearrange("p (t j) -> p t j", j=EAD))
            nc.scalar.dma_start(out=ea_t[:, 2 * T:, :],
                                in_=ea_d[:, 2 * T * EAD:].rearrange("p (t j) -> p t j", j=EAD))
            at_t = singles.tile([128, ST, NPP], at_dt)

            ht = singles.tile([128, ST, CH], fp16)
            aggT_all = singles.tile([128, 4, NPP], fp16)
            eaT_all = singles.tile([EAD, WPC, 128], fp16)
            eaw_t = singles.tile([128, WPC, EAD], fp16)
            recip_all = singles.tile([128, WPC], f32)

            zinA = [dram.tile([NHA // N_CORES, CH], fp16, name=f"zinA{l}", tag=f"zinA{l}")
                    for l in range(3)]
            zinB = [dram.tile([NHB // N_CORES, CH], fp16, name=f"zinB{l}", tag=f"zinB{l}")
                    for l in range(3)]
            zfullA = [dram.tile([NHA, CH], fp16, name=f"zfullA{l}", tag=f"zfullA{l}",
                                addr_space="Shared")
                      for l in range(3)]
            zfullB = [dram.tile([NHB, CH], fp16, name=f"zfullB{l}", tag=f"zfullB{l}",
                                addr_space="Shared")
                      for l in range(3)]

            def emit_z(p, w):
                """z matmuls + scale + publish for window w of product p."""
                psum_z = ps_z.tile([128, 128], f32, tag="z", name="psum_z")
                for i, (m, kk) in enumerate(PIECES[p]):
                    nc.tensor.matmul(psum_z[:, :],
                                     lhsT=aggT_all[:, m, 128 * w:128 * (w + 1)],
                                     rhs=wx_t[:, kk, :], start=(i == 0), stop=False)
                nc.tensor.matmul(psum_z[:, :], lhsT=eaT_all[:, w, :],
                                 rhs=wep_t[:, p, :], start=False, stop=True)
                last = p == _NPROD - 1
                z_t = zpool.tile([128, 128], f32 if last else fp16,
                                 tag="z32" if last else "z16", name="z_t")
                nc.vector.tensor_scalar(
                    out=z_t[:, :], in0=psum_z[:, :],
                    scalar1=recip_all[:, w:w + 1], scalar2=None,
                    op0=mybir.AluOpType.mult,
                )
                if last:
                    wsz = wsizes[w]
                    nc.sync.dma_start(out=out_d[128 * w:128 * w + wsz, :],
                                      in_=z_t[:wsz, :])
                elif w < WA:
                    nc.sync.dma_start(out=zinA[p][128 * w:128 * (w + 1), :], in_=z_t[:, :])
                else:
                    nc.sync.dma_start(out=zinB[p][128 * (w - WA):128 * (w - WA + 1), :],
                                      in_=z_t[:, :])

            for p in range(_NPROD):
                if p == 0:
                    pass
                else:
                    for i in range(0, STA, 8):
                        j = min(i + 8, STA)
                        nc.sync.dma_start(
                            out=ht[:, i:j, :],
                            in_=zfullA[p - 1][128 * i:128 * j, :]
                            .rearrange("(s q) c -> q s c", q=128))
                    for i in range(STA, ST, 8):
                        j = min(i + 8, ST)
                        nc.sync.dma_start(
                            out=ht[:, i:j, :],
                            in_=zfullB[p - 1][128 * (i - STA):128 * (j - STA), :]
                            .rearrange("(s q) c -> q s c", q=128))

                def emit_ea(w):
                    # psum_ea[dst, k] accumulates P_t^T @ ea_t for window w.
                    p_t = ppool.tile([128, T * 128], mybir.dt.float8e3, tag="p", name="p_t")
                    nc.scalar.dma_start(out=p_t[:, :],
                                        in_=p_d[:, w * T * 128:(w + 1) * T * 128])
                    psum_ea = ps_ea.tile([128, EAD], f32, tag="eaw", name="psum_ea")
                    for t in range(T):
                        nc.tensor.matmul(psum_ea[:, :],
                                         lhsT=p_t[:, t * 128:(t + 1) * 128],
                                         rhs=ea_t[:, w * T + t, :],
                                         start=(t == 0), stop=(t == T - 1))
                    nc.vector.tensor_copy(out=eaw_t[:, w, :], in_=psum_ea[:, :])
                    den_t = small.tile([128, 1], f32, tag="den", name="den_t")
                    nc.vector.tensor_scalar_max(den_t[:, :], psum_ea[:, EDGE_DIM:EAD], 1.0)
                    nc.vector.reciprocal(recip_all[:, w:w + 1], den_t[:, :])
                    # transpose [128, EAD] -> [EAD, 128] via PE
                    psum_eaT = ps_ea.tile([EAD, 128], fp16, tag="eaTp", name="psum_eaT")
                    nc.tensor.matmul(psum_eaT[:, :], lhsT=eaw_t[:, w, :],
                                     rhs=id_t[:, :], start=True, stop=True,
                                     is_transpose=True)
                    nc.vector.tensor_copy(out=eaT_all[:, w, :], in_=psum_eaT[:, :])

                if p == 0:
                    for w in CHUNK_WINS[0]:
                        emit_ea(w)
                    # p0-critical DMA order on the scalar ring: x, then at
                    # (A-src tiles first) — all behind ea + P w0-3.
                    nc.scalar.dma_start(out=ht[:, :, :],
                                        in_=x16[:, :].rearrange("q (s c) -> q s c", c=CH))
                    for ci in range(10):
                        nc.scalar.dma_start(out=at_t[:, 8 * ci:8 * (ci + 1), :],
                                            in_=at_d[:, 8 * ci:8 * (ci + 1), :])

                # dense aggregation: aggT[ch, dst] = sum_s H_s^T @ AT_s
                psk = [ps_agg.tile([128, b - a], f32, tag=f"agg{k}", name=f"agg{k}")
                       for k, (a, b) in enumerate(CHUNKS)]

                def agg_mm(s, k, start, stop):
                    a, b = CHUNKS[k]
                    nc.tensor.matmul(psk[k][:, :], lhsT=ht[:, s, :],
                                     rhs=at_t[:, s, a:b], start=start, stop=stop)

                # A-block: all chunks over A srcs
                for s in range(STA):
                    for k in range(3):
                        agg_mm(s, k, start=(s == 0), stop=False)
                # B chunk0, then publish A windows
                for s in range(STA, ST):
                    agg_mm(s, 0, start=False, stop=(s == ST - 1))
                nc.vector.tensor_copy(out=aggT_all[:, p, CHUNKS[0][0]:CHUNKS[0][1]],
                                      in_=psk[0][:, :])
                for w in CHUNK_WINS[0]:
                    emit_z(p, w)
                if p < _NCC and p < _NPROD - 1:
                    nc.gpsimd.collective_compute(
                        "AllGather", mybir.AluOpType.bypass,
                        replica_groups=[list(range(N_CORES))],
                        ins=[zinA[p].opt()], outs=[zfullA[p].opt()],
                    )
                # B chunk1, publish w4-7
                for s in range(STA, ST):
                    agg_mm(s, 1, start=False, stop=(s == ST - 1))
                if p == 0:
                    for w in CHUNK_WINS[1]:
                        emit_ea(w)
                nc.vector.tensor_copy(out=aggT_all[:, p, CHUNKS[1][0]:CHUNKS[1][1]],
                                      in_=psk[1][:, :])
                for w in CHUNK_WINS[1]:
                    emit_z(p, w)
                # B chunk2, publish w8-9
                for s in range(STA, ST):
                    agg_mm(s, 2, start=False, stop=(s == ST - 1))
                if p == 0:
                    for w in CHUNK_WINS[2]:
                        emit_ea(w)
                nc.vector.tensor_copy(out=aggT_all[:, p, CHUNKS[2][0]:CHUNKS[2][1]],
                                      in_=psk[2][:, :])
                for w in CHUNK_WINS[2]:
                    emit_z(p, w)
                if p < _NCC and p < _NPROD - 1:
                    nc.gpsimd.collective_compute(
                        "AllGather", mybir.AluOpType.bypass,
                        replica_groups=[list(range(N_CORES))],
                        ins=[zinB[p].opt()], outs=[zfullB[p].opt()],
                    )
    nc.finalize()
    return nc


_CACHE = {}


def _get_program(T, use_fp8):
    key = (T, use_fp8)
    if key not in _CACHE:
        from concourse import mybir, bacc
        import concourse.bass as bass
        import concourse.tile as tile
        _CACHE[key] = _build(T, use_fp8, mybir, bass, tile, bacc)
    return _CACHE[key]


def _run(inputs, trace=False, tmpdir=None):
    from concourse.bass_utils import run_bass_kernel_spmd

    x = np.asarray(inputs["x"], np.float32)
    edge_attr = np.asarray(inputs["edge_attr"], np.float32)
    edge_index = np.asarray(inputs["edge_index"])
    Ws = [np.asarray(inputs[f"W{i}"], np.float32) for i in range(4)]
    bs = [np.asarray(inputs[f"b{i}"], np.float32) for i in range(4)]

    # fp8e3 (e3m4) holds integers 0..15 exactly; fall back to fp16 in the
    # (practically impossible) case of a >15-fold repeated edge.
    src_i = np.asarray(edge_index[0], np.int64)
    dst_i = np.asarray(edge_index[1], np.int64)
    _, pair_counts = np.unique(src_i * N_NODES + dst_i, return_counts=True)
    max_mult = int(pair_counts.max())
    # fp8e3 (e3m4) holds integers 0..15 exactly; a >15-fold repeated edge is
    # probabilistically impossible for these inputs, and the fp16 adjacency
    # would not fit SBUF, so fail loudly rather than compute wrong sums.
    assert _ATDT != "fp8" or max_mult <= 15,         f"edge multiplicity {max_mult} exceeds fp8e3 exact range"
    use_fp8 = _ATDT == "fp8"
    at_np_dt = ml_dtypes.float8_e3m4 if use_fp8 else np.float16
    at, p_pk, ea_pk, T = _prep(edge_index, edge_attr, at_np_dt)
    nc = _get_program(T, use_fp8)

    # x in padded, PERM-tile layout
    xp = np.zeros((NPAD, CH), np.float16)
    for c in range(N_CORES):
        xp[NPP * c:NPP * c + NPC] = x[NPC * c:NPC * (c + 1)].astype(np.float16)
    x16 = xp.reshape(ST, 128, CH)[PERM].transpose(1, 0, 2).reshape(128, ST * CH)

    wx = np.stack([
        Ws[0][:128], Ws[1][:128],
        Ws[2][:128], Ws[2][128:256],
        Ws[3][:128], Ws[3][128:256], Ws[3][256:384],
    ]).astype(np.float16)
    Cs = [128, 128, 256, 384]
    wep = np.stack([
        np.concatenate([Ws[l][Cs[l]:Cs[l] + EDGE_DIM], bs[l][None, :]], axis=0)
        for l in range(4)
    ]).astype(np.float16)

    NT = WPC * T
    in_maps = []
    for c in range(N_CORES):
        in_maps.append({
            "x16": x16,
            "at": at[c],
            "pt": p_pk[c],
            "ea": ea_pk[c].reshape(128, NT * EAD),
            "wx": wx,
            "wep": wep,
            "id128": np.eye(128, dtype=np.float16),
        })
    res = run_bass_kernel_spmd(nc, in_maps, core_ids=list(range(N_CORES)),
                               trace=trace, tmpdir=tmpdir)
    out = np.concatenate([res.results[c]["zout"] for c in range(N_CORES)], axis=0)
    return out, res


def kernel(**inputs) -> np.ndarray:
    out, _ = _run(inputs, trace=False)
    return out



# revision 6
# speedup vs baseline: 1.1385x; 1.1385x over previous
"""DenseGCNBlock on 8 trn2 NeuronCores (Bass/Tile) — dense-adjacency version.

Math (per layer l, weight W_l [C_l+16, 128]):
    msg_e = concat(cat[src_e], ea_e) @ W_l + b_l
    z_l   = segment_sum(msg, dst) / max(counts, 1)
Linearity splits z_l into blocks:
    z_l = (sum_m agg_m @ Wx_block_m + eaT^T @ [We_l;b_l]) / denom
where agg_m = H_m^T @ A is the dense-adjacency aggregation of feature block m
and eaT = [segment_sum(ea); counts] is computed on HOST (it depends only on
edge data, which host prep already sorts/buckets).

Aggregation: aggT[ch, dst] = sum_s H_s^T @ AT_s over ST=80 src tiles, with
AT [10240 padded src, 1250 real dst per core] host-built fp8e3 (exact small
ints), stored CHUNK-MAJOR (three contiguous blocks of 512/512/226 dst cols)
and RESIDENT in SBUF.

Schedule: node tiles permuted so A half = windows 0-3 of every core (32 src
tiles), B half = windows 4-9 (48 tiles).
  p0 (chunk-major, x is local):   chunk0 over all 80 tiles -> z w0-3 ->
    AllGather-A(0) early; chunk1 -> z w4-7; chunk2 -> z w8-9 -> AllGather-B(0).
  p1..p3 (A-block first): [A srcs x chunks 0,1,2][B chunk0] z w0-3 -> AG-A;
    [B chunk1] z w4-7; [B chunk2] z w8-9 -> AG-B.
ht is double-buffered so the next product's reload overlaps current compute.
A tiny warm-up AllGather at t~0 absorbs cross-core launch skew so the first
real collective runs warm.
"""
import os
import sys

sys.path.insert(0, "/opt/trn_rl_repo")

import numpy as np
import ml_dtypes

_NPROD = int(os.environ.get("KERNEL_NPROD", "4"))  # debug knob: products to run
_NCC = int(os.environ.get("KERNEL_NCC", "3"))      # debug knob: collectives to run

N_NODES = 10000
N_EDGES = 320000
CH = 128
EDGE_DIM = 16
EAD = EDGE_DIM + 1  # 16 attrs + counts row
N_CORES = 8
NPC = N_NODES // N_CORES   # 1250 real nodes per core
WPC = 10                   # windows per core
WA = 4                     # windows in the A half
NPP = WPC * 128            # 1280 padded nodes per core
NPAD = N_CORES * NPP       # 10240 padded nodes total
ST = NPAD // 128           # 80 src tiles
STA = N_CORES * WA         # 32 src tiles in the A half
NHA = STA * 128            # 4096 nodes in the A half
NHB = NPAD - NHA           # 6144 nodes in the B half
CHUNKS = ((0, 512), (512, 1024), (1024, NPC))  # dst col blocks (real cols only)
CHUNK_WINS = ((0, 1, 2, 3), (4, 5, 6, 7), (8, 9))

# node-tile permutation: A half = (c, w<WA), B half = (c, w>=WA)
PERM = [10 * c + w for c in range(N_CORES) for w in range(WA)] + \
       [10 * c + w for c in range(N_CORES) for w in range(WA, WPC)]


def _prep(edge_index, edge_attr):
    """Sort edges by dst; build per-core dense (padded, tile-permuted,
    chunk-major) adjacency blocks, plus host-side EA aggregation + recip."""
    src = np.asarray(edge_index[0], dtype=np.int64)
    dst = np.asarray(edge_index[1], dtype=np.int64)
    ea = np.asarray(edge_attr, dtype=np.float32)

    order = np.argsort(dst, kind="stable")
    src_s = src[order]
    dst_s = dst[order]
    ea_s = ea[order]
    spad_s = (src_s // NPC) * NPP + (src_s % NPC)  # padded global src ids

    # host EA aggregation: seg[n, :16] = sum of edge attrs into n, seg[n, 16] = count
    ea_aug = np.concatenate([ea_s, np.ones((ea_s.shape[0], 1), np.float32)], axis=1)
    csum = np.zeros((ea_s.shape[0] + 1, EAD), np.float64)
    csum[1:] = np.cumsum(ea_aug.astype(np.float64), axis=0)
    bounds = np.searchsorted(dst_s, np.arange(N_NODES + 1))
    seg = (csum[bounds[1:]] - csum[bounds[:-1]]).astype(np.float32)  # [N, 17]

    at = []
    eaT = np.zeros((N_CORES, EAD, WPC * 128), np.float16)
    recip = np.zeros((N_CORES, 128, WPC), np.float32)
    for c in range(N_CORES):
        lo, hi = bounds[NPC * c], bounds[NPC * (c + 1)]
        flat = spad_s[lo:hi] * NPC + (dst_s[lo:hi] - NPC * c)
        cnt = np.bincount(flat, minlength=NPAD * NPC)
        atc = cnt.astype(np.float32).reshape(ST, 128, NPC)[PERM]
        atc = atc.transpose(1, 0, 2)  # [128, ST, NPC]
        at.append([np.ascontiguousarray(atc[:, :, a:b]).astype(ml_dtypes.float8_e3m4)
                   for a, b in CHUNKS])
        segc = np.zeros((NPP, EAD), np.float32)
        segc[:NPC] = seg[NPC * c:NPC * (c + 1)]
        eaT[c] = segc.reshape(WPC, 128, EAD).transpose(2, 0, 1).reshape(EAD, WPC * 128)
        den = np.maximum(segc[:, EDGE_DIM], 1.0)
        recip[c] = (1.0 / den).reshape(WPC, 128).T
    return at, eaT, recip


def _build(mybir, bass, tile, bacc):
    """Build the SPMD Bass program (same for all cores)."""
    fp16 = mybir.dt.float16
    f32 = mybir.dt.float32
    at_dt = mybir.dt.float8e3

    nc = bacc.Bacc("TRN2", num_devices=N_CORES)
    # x16 already in permuted-tile layout
    x16 = nc.dram_tensor("x16", [128, ST * CH], fp16, kind="ExternalInput")
    at_d = [nc.dram_tensor(f"at{k}", [128, ST, b - a], at_dt, kind="ExternalInput")
            for k, (a, b) in enumerate(CHUNKS)]
    ea_d = nc.dram_tensor("eaT", [EAD, WPC * 128], fp16, kind="ExternalInput")
    recip_d = nc.dram_tensor("recip", [128, WPC], f32, kind="ExternalInput")
    wx_d = nc.dram_tensor("wx", [7, 128, 128], fp16, kind="ExternalInput")
    wep_d = nc.dram_tensor("wep", [4, EAD, 128], fp16, kind="ExternalInput")
    out_d = nc.dram_tensor("zout", [NPC, CH], f32, kind="ExternalOutput")

    PIECES = {0: [(0, 0)], 1: [(1, 1)], 2: [(1, 2), (2, 3)], 3: [(1, 4), (2, 5), (3, 6)]}
    wsizes = [128] * (WPC - 1) + [NPC - 128 * (WPC - 1)]
    PAD9 = 128 - wsizes[WPC - 1]  # pad rows in window 9

    with tile.TileContext(nc) as tc:
        with tc.tile_pool(name="singles", bufs=1) as singles, \
             tc.tile_pool(name="hpool", bufs=2) as hpool, \
             tc.tile_pool(name="zpool", bufs=4) as zpool, \
             tc.tile_pool(name="ps_agg", bufs=1, space="PSUM") as ps_agg, \
             tc.tile_pool(name="ps_z", bufs=3, space="PSUM") as ps_z, \
             tc.tile_pool(name="dram", bufs=1, space="DRAM") as dram:

            # ---- warm-up collective (absorbs launch skew, warms CC path) ----
            warm_in = dram.tile([8, 16], fp16, name="warm_in", tag="warm_in")
            warm_out = dram.tile([8 * N_CORES, 16], fp16, name="warm_out",
                                 tag="warm_out", addr_space="Shared")
            warm_s = singles.tile([8, 16], fp16)
            nc.vector.memset(warm_s[:, :], 0.0)
            nc.sync.dma_start(out=warm_in[:, :], in_=warm_s[:, :])
            nc.gpsimd.collective_compute(
                "AllGather", mybir.AluOpType.bypass,
                replica_groups=[list(range(N_CORES))],
                ins=[warm_in.opt()], outs=[warm_out.opt()],
            )

            # ---- small singles (scalar ring, ahead of the at blocks) ----
            wx_t = singles.tile([128, 7, 128], fp16)
            nc.scalar.dma_start(out=wx_t[:, :, :], in_=wx_d[:, :, :].rearrange("k p j -> p k j"))
            wep_t = singles.tile([EAD, 4, 128], fp16)
            nc.scalar.dma_start(out=wep_t[:, :, :], in_=wep_d[:, :, :].rearrange("l p j -> p l j"))
            eaT_t = singles.tile([EAD, WPC, 128], fp16)
            nc.scalar.dma_start(out=eaT_t[:, :, :],
                                in_=ea_d[:, :].rearrange("p (w j) -> p w j", j=128))
            recip_all = singles.tile([128, WPC], f32)
            nc.scalar.dma_start(out=recip_all[:, :], in_=recip_d[:, :])

            at_t = [singles.tile([128, ST, b - a], at_dt, name=f"at_t{k}", tag=f"at_t{k}")
                    for k, (a, b) in enumerate(CHUNKS)]
            aggT_all = singles.tile([128, 4, NPP], fp16)

            zinA = [dram.tile([NHA // N_CORES, CH], fp16, name=f"zinA{l}", tag=f"zinA{l}")
                    for l in range(3)]
            zinB = [dram.tile([NHB // N_CORES, CH], fp16, name=f"zinB{l}", tag=f"zinB{l}")
                    for l in range(3)]
            zfullA = [dram.tile([NHA, CH], fp16, name=f"zfullA{l}", tag=f"zfullA{l}",
                                addr_space="Shared")
                      for l in range(3)]
            zfullB = [dram.tile([NHB, CH], fp16, name=f"zfullB{l}", tag=f"zfullB{l}",
                                addr_space="Shared")
                      for l in range(3)]

            # zero the pad rows of window 9 in each zinB once (their gathered
            # values become ht rows; the matching at rows are all-zero, but the
            # values must be finite)
            zpad = singles.tile([PAD9, CH], fp16)
            nc.vector.memset(zpad[:, :], 0.0)
            w9lo = 128 * (WPC - 1 - WA) + wsizes[WPC - 1]
            for l in range(min(3, _NCC)):
                nc.sync.dma_start(out=zinB[l][w9lo:w9lo + PAD9, :], in_=zpad[:, :])

            def emit_z(p, w):
                """z matmuls + scale + publish for window w of product p."""
                ww = wsizes[w]
                psum_z = ps_z.tile([128, 128], f32, tag="z", name="psum_z")
                for i, (m, kk) in enumerate(PIECES[p]):
                    nc.tensor.matmul(psum_z[:ww, :],
                                     lhsT=aggT_all[:, m, 128 * w:128 * w + ww],
                                     rhs=wx_t[:, kk, :], start=(i == 0), stop=False)
                nc.tensor.matmul(psum_z[:ww, :], lhsT=eaT_t[:, w, :ww],
                                 rhs=wep_t[:, p, :], start=False, stop=True)
                last = p == _NPROD - 1
                z_t = zpool.tile([128, 128], f32 if last else fp16,
                                 tag="z32" if last else "z16", name="z_t")
                nc.vector.tensor_scalar(
                    out=z_t[:ww, :], in0=psum_z[:ww, :],
                    scalar1=recip_all[:ww, w:w + 1], scalar2=None,
                    op0=mybir.AluOpType.mult,
                )
                if last:
                    nc.sync.dma_start(out=out_d[128 * w:128 * w + ww, :],
                                      in_=z_t[:ww, :])
                elif w < WA:
                    nc.sync.dma_start(out=zinA[p][128 * w:128 * (w + 1), :], in_=z_t[:, :])
                else:
                    nc.sync.dma_start(out=zinB[p][128 * (w - WA):128 * (w - WA) + ww, :],
                                      in_=z_t[:ww, :])

            def publish(p, k, psk):
                a, b = CHUNKS[k]
                nc.vector.tensor_copy(out=aggT_all[:, p, a:b], in_=psk[:, :])
                for w in CHUNK_WINS[k]:
                    emit_z(p, w)
                if p < _NCC and p < _NPROD - 1:
                    if k == 0:
                        nc.gpsimd.collective_compute(
                            "AllGather", mybir.AluOpType.bypass,
                            replica_groups=[list(range(N_CORES))],
                            ins=[zinA[p].opt()], outs=[zfullA[p].opt()],
                        )
                    elif k == 2:
                        nc.gpsimd.collective_compute(
                            "AllGather", mybir.AluOpType.bypass,
                            replica_groups=[list(range(N_CORES))],
                            ins=[zinB[p].opt()], outs=[zfullB[p].opt()],
                        )

            for p in range(_NPROD):
                ht = hpool.tile([128, ST, CH], fp16, tag="ht", name="ht")
                if p == 0:
                    # x16 on the sync ring; at blocks stream on scalar ring
                    for i in range(0, ST, 8):
                        nc.sync.dma_start(
                            out=ht[:, i:i + 8, :],
                            in_=x16[:, i * CH:(i + 8) * CH]
                            .rearrange("q (s c) -> q s c", c=CH))
                    for k in range(3):
                        for i in range(0, ST, 8):
                            nc.scalar.dma_start(out=at_t[k][:, i:i + 8, :],
                                                in_=at_d[k][:, i:i + 8, :])
                else:
                    for i in range(0, STA, 8):
                        nc.sync.dma_start(
                            out=ht[:, i:i + 8, :],
                            in_=zfullA[p - 1][128 * i:128 * (i + 8), :]
                            .rearrange("(s q) c -> q s c", q=128))
                    for i in range(STA, ST, 8):
                        nc.sync.dma_start(
                            out=ht[:, i:i + 8, :],
                            in_=zfullB[p - 1][128 * (i - STA):128 * (i - STA + 8), :]
                            .rearrange("(s q) c -> q s c", q=128))

                psk = [ps_agg.tile([128, b - a], f32, tag=f"agg{k}", name=f"agg{k}")
                       for k, (a, b) in enumerate(CHUNKS)]

                def agg_mm(s, k, start, stop):
                    nc.tensor.matmul(psk[k][:, :], lhsT=ht[:, s, :],
                                     rhs=at_t[k][:, s, :], start=start, stop=stop)

                if p == 0:
                    # chunk-major: publish each chunk as early as possible
                    for k in range(3):
                        for s in range(ST):
                            agg_mm(s, k, start=(s == 0), stop=(s == ST - 1))
                        publish(p, k, psk[k])
                else:
                    # A-block: all chunks over A srcs (gathered first)
                    for s in range(STA):
                        for k in range(3):
                            agg_mm(s, k, start=(s == 0), stop=False)
                    # B chunk0, then publish A windows + AllGather-A
                    for s in range(STA, ST):
                        agg_mm(s, 0, start=False, stop=(s == ST - 1))
                    publish(p, 0, psk[0])
                    for s in range(STA, ST):
                        agg_mm(s, 1, start=False, stop=(s == ST - 1))
                    publish(p, 1, psk[1])
                    for s in range(STA, ST):
                        agg_mm(s, 2, start=False, stop=(s == ST - 1))
                    publish(p, 2, psk[2])
    nc.finalize()
    return nc


_CACHE = {}


def _get_program():
    if "nc" not in _CACHE:
        from concourse import mybir, bacc
        import concourse.bass as bass
        import concourse.tile as tile
        _CACHE["nc"] = _build(mybir, bass, tile, bacc)
    return _CACHE["nc"]


def _run(inputs, trace=False, tmpdir=None):
    from concourse.bass_utils import run_bass_kernel_spmd

    x = np.asarray(inputs["x"], np.float32)
    edge_attr = np.asarray(inputs["edge_attr"], np.float32)
    edge_index = np.asarray(inputs["edge_index"])
    Ws = [np.asarray(inputs[f"W{i}"], np.float32) for i in range(4)]
    bs = [np.asarray(inputs[f"b{i}"], np.float32) for i in range(4)]

    # fp8e3 (e3m4) holds integers 0..15 exactly; a >15-fold repeated edge is
    # probabilistically impossible for these inputs, so fail loudly rather
    # than compute wrong sums.
    src_i = np.asarray(edge_index[0], np.int64)
    dst_i = np.asarray(edge_index[1], np.int64)
    _, pair_counts = np.unique(src_i * N_NODES + dst_i, return_counts=True)
    assert int(pair_counts.max()) <= 15, \
        f"edge multiplicity {int(pair_counts.max())} exceeds fp8e3 exact range"
    at, eaT, recip = _prep(edge_index, edge_attr)
    nc = _get_program()

    # x in padded, PERM-tile layout
    xp = np.zeros((NPAD, CH), np.float16)
    for c in range(N_CORES):
        xp[NPP * c:NPP * c + NPC] = x[NPC * c:NPC * (c + 1)].astype(np.float16)
    x16 = xp.reshape(ST, 128, CH)[PERM].transpose(1, 0, 2).reshape(128, ST * CH)

    wx = np.stack([
        Ws[0][:128], Ws[1][:128],
        Ws[2][:128], Ws[2][128:256],
        Ws[3][:128], Ws[3][128:256], Ws[3][256:384],
    ]).astype(np.float16)
    Cs = [128, 128, 256, 384]
    wep = np.stack([
        np.concatenate([Ws[l][Cs[l]:Cs[l] + EDGE_DIM], bs[l][None, :]], axis=0)
        for l in range(4)
    ]).astype(np.float16)

    in_maps = []
    for c in range(N_CORES):
        m = {
            "x16": x16,
            "eaT": eaT[c],
            "recip": recip[c],
            "wx": wx,
            "wep": wep,
        }
        for k in range(3):
            m[f"at{k}"] = at[c][k]
        in_maps.append(m)
    res = run_bass_kernel_spmd(nc, in_maps, core_ids=list(range(N_CORES)),
                               trace=trace, tmpdir=tmpdir)
    out = np.concatenate([res.results[c]["zout"] for c in range(N_CORES)], axis=0)
    return out, res


def kernel(**inputs) -> np.ndarray:
    out, _ = _run(inputs, trace=False)
    return out


# revision 8
# speedup vs baseline: 1.2010x; 1.0549x over previous
"""DenseGCNBlock on 8 trn2 NeuronCores (Bass/Tile) — dense-adjacency version.

Math (per layer l, weight W_l [C_l+16, 128]):
    msg_e = concat(cat[src_e], ea_e) @ W_l + b_l
    z_l   = segment_sum(msg, dst) / max(counts, 1)
Linearity splits z_l into blocks:
    z_l = (sum_m agg_m @ Wx_block_m + eaT^T @ [We_l;b_l]) / denom
where agg_m = H_m^T @ A is the dense-adjacency aggregation of feature block m
and eaT = [segment_sum(ea); counts] is computed on HOST (it depends only on
edge data, which host prep already sorts/buckets).

Aggregation: aggT[ch, dst] = sum_s H_s^T @ AT_s over ST=80 src tiles, with
AT [10240 padded src, 1250 real dst per core] host-built fp8e3 (exact small
ints), stored CHUNK-MAJOR (three contiguous blocks of 512/512/226 dst cols)
and RESIDENT in SBUF.

Schedule: node tiles permuted so A half = windows 0-3 of every core (32 src
tiles), B half = windows 4-9 (48 tiles).
  p0 (chunk-major, x is local):   chunk0 over all 80 tiles -> z w0-3 ->
    AllGather-A(0) early; chunk1 -> z w4-7; chunk2 -> z w8-9 -> AllGather-B(0).
  p1..p3 (A-block first): [A srcs x chunks 0,1,2][B chunk0] z w0-3 -> AG-A;
    [B chunk1] z w4-7; [B chunk2] z w8-9 -> AG-B.
ht is double-buffered so the next product's reload overlaps current compute.
A tiny warm-up AllGather at t~0 absorbs cross-core launch skew so the first
real collective runs warm.
"""
import os
import sys

sys.path.insert(0, "/opt/trn_rl_repo")

import numpy as np
import ml_dtypes

_NPROD = int(os.environ.get("KERNEL_NPROD", "4"))  # debug knob: products to run
_NCC = int(os.environ.get("KERNEL_NCC", "3"))      # debug knob: collectives to run

N_NODES = 10000
N_EDGES = 320000
CH = 128
EDGE_DIM = 16
EAD = EDGE_DIM + 1  # 16 attrs + counts row
N_CORES = 8
NPC = N_NODES // N_CORES   # 1250 real nodes per core
WPC = 10                   # windows per core
WA = 4                     # windows in the A half
NPP = WPC * 128            # 1280 padded nodes per core
NPAD = N_CORES * NPP       # 10240 padded nodes total
ST = NPAD // 128           # 80 src tiles
STA = N_CORES * WA         # 32 src tiles in the A half
NHA = STA * 128            # 4096 nodes in the A half
NHB = NPAD - NHA           # 6144 nodes in the B half
CHUNKS = ((0, 512), (512, 1024), (1024, NPC))  # dst col blocks (real cols only)
CHUNK_WINS = ((0, 1, 2, 3), (4, 5, 6, 7), (8, 9))

# node-tile permutation: A half = (c, w<WA), B half = (c, w>=WA)
PERM = [10 * c + w for c in range(N_CORES) for w in range(WA)] + \
       [10 * c + w for c in range(N_CORES) for w in range(WA, WPC)]


def _prep(edge_index, edge_attr):
    """Sort edges by dst; build per-core dense (padded, tile-permuted,
    chunk-major) adjacency blocks, plus host-side EA aggregation + recip."""
    src = np.asarray(edge_index[0], dtype=np.int64)
    dst = np.asarray(edge_index[1], dtype=np.int64)
    ea = np.asarray(edge_attr, dtype=np.float32)

    order = np.argsort(dst, kind="stable")
    src_s = src[order]
    dst_s = dst[order]
    ea_s = ea[order]
    spad_s = (src_s // NPC) * NPP + (src_s % NPC)  # padded global src ids

    # host EA aggregation: seg[n, :16] = sum of edge attrs into n, seg[n, 16] = count
    ea_aug = np.concatenate([ea_s, np.ones((ea_s.shape[0], 1), np.float32)], axis=1)
    csum = np.zeros((ea_s.shape[0] + 1, EAD), np.float64)
    csum[1:] = np.cumsum(ea_aug.astype(np.float64), axis=0)
    bounds = np.searchsorted(dst_s, np.arange(N_NODES + 1))
    seg = (csum[bounds[1:]] - csum[bounds[:-1]]).astype(np.float32)  # [N, 17]

    at = []
    eaT = np.zeros((N_CORES, EAD, WPC * 128), np.float16)
    recip = np.zeros((N_CORES, 128, WPC), np.float32)
    for c in range(N_CORES):
        lo, hi = bounds[NPC * c], bounds[NPC * (c + 1)]
        flat = spad_s[lo:hi] * NPC + (dst_s[lo:hi] - NPC * c)
        cnt = np.bincount(flat, minlength=NPAD * NPC)
        atc = cnt.astype(np.float32).reshape(ST, 128, NPC)[PERM]
        atc = atc.transpose(1, 0, 2)  # [128, ST, NPC]
        at.append([np.ascontiguousarray(atc[:, :, a:b]).astype(ml_dtypes.float8_e3m4)
                   for a, b in CHUNKS])
        segc = np.zeros((NPP, EAD), np.float32)
        segc[:NPC] = seg[NPC * c:NPC * (c + 1)]
        eaT[c] = segc.reshape(WPC, 128, EAD).transpose(2, 0, 1).reshape(EAD, WPC * 128)
        den = np.maximum(segc[:, EDGE_DIM], 1.0)
        recip[c] = (1.0 / den).reshape(WPC, 128).T
    return at, eaT, recip


def _build(mybir, bass, tile, bacc):
    """Build the SPMD Bass program (same for all cores)."""
    fp16 = mybir.dt.float16
    f32 = mybir.dt.float32
    at_dt = mybir.dt.float8e3

    nc = bacc.Bacc("TRN2", num_devices=N_CORES)
    # x16 already in permuted-tile layout
    x16 = nc.dram_tensor("x16", [128, ST * CH], fp16, kind="ExternalInput")
    at_d = [nc.dram_tensor(f"at{k}", [128, ST, b - a], at_dt, kind="ExternalInput")
            for k, (a, b) in enumerate(CHUNKS)]
    ea_d = nc.dram_tensor("eaT", [EAD, WPC * 128], fp16, kind="ExternalInput")
    recip_d = nc.dram_tensor("recip", [128, WPC], f32, kind="ExternalInput")
    wx_d = nc.dram_tensor("wx", [7, 128, 128], fp16, kind="ExternalInput")
    wep_d = nc.dram_tensor("wep", [4, EAD, 128], fp16, kind="ExternalInput")
    out_d = nc.dram_tensor("zout", [NPC, CH], f32, kind="ExternalOutput")

    PIECES = {0: [(0, 0)], 1: [(1, 1)], 2: [(1, 2), (2, 3)], 3: [(1, 4), (2, 5), (3, 6)]}
    wsizes = [128] * (WPC - 1) + [NPC - 128 * (WPC - 1)]
    PAD9 = 128 - wsizes[WPC - 1]  # pad rows in window 9

    with tile.TileContext(nc) as tc:
        with tc.tile_pool(name="singles", bufs=1) as singles, \
             tc.tile_pool(name="hpool", bufs=2) as hpool, \
             tc.tile_pool(name="zpool", bufs=4) as zpool, \
             tc.tile_pool(name="ps_agg", bufs=1, space="PSUM") as ps_agg, \
             tc.tile_pool(name="ps_z", bufs=3, space="PSUM") as ps_z, \
             tc.tile_pool(name="dram", bufs=1, space="DRAM") as dram:

            # ---- small singles (scalar ring, ahead of the at blocks) ----
            wx_t = singles.tile([128, 7, 128], fp16)
            nc.scalar.dma_start(out=wx_t[:, :, :], in_=wx_d[:, :, :].rearrange("k p j -> p k j"))
            wep_t = singles.tile([EAD, 4, 128], fp16)
            nc.scalar.dma_start(out=wep_t[:, :, :], in_=wep_d[:, :, :].rearrange("l p j -> p l j"))
            eaT_t = singles.tile([EAD, WPC, 128], fp16)
            nc.scalar.dma_start(out=eaT_t[:, :, :],
                                in_=ea_d[:, :].rearrange("p (w j) -> p w j", j=128))
            recip_all = singles.tile([128, WPC], f32)
            nc.scalar.dma_start(out=recip_all[:, :], in_=recip_d[:, :])

            at_t = [singles.tile([128, ST, b - a], at_dt, name=f"at_t{k}", tag=f"at_t{k}")
                    for k, (a, b) in enumerate(CHUNKS)]
            aggT_all = singles.tile([128, 4, NPP], fp16)

            zinA = [dram.tile([NHA // N_CORES, CH], fp16, name=f"zinA{l}", tag=f"zinA{l}")
                    for l in range(3)]
            zinB = [dram.tile([NHB // N_CORES, CH], fp16, name=f"zinB{l}", tag=f"zinB{l}")
                    for l in range(3)]
            zfullA = [dram.tile([NHA, CH], fp16, name=f"zfullA{l}", tag=f"zfullA{l}",
                                addr_space="Shared")
                      for l in range(3)]
            zfullB = [dram.tile([NHB, CH], fp16, name=f"zfullB{l}", tag=f"zfullB{l}",
                                addr_space="Shared")
                      for l in range(3)]

            # zero the pad rows of window 9 in each zinB once (their gathered
            # values become ht rows; the matching at rows are all-zero, but the
            # values must be finite)
            zpad = singles.tile([PAD9, CH], fp16)
            nc.vector.memset(zpad[:, :], 0.0)
            w9lo = 128 * (WPC - 1 - WA) + wsizes[WPC - 1]
            for l in range(min(3, _NCC)):
                nc.sync.dma_start(out=zinB[l][w9lo:w9lo + PAD9, :], in_=zpad[:, :])

            def emit_z(p, w):
                """z matmuls + scale + publish for window w of product p."""
                ww = wsizes[w]
                psum_z = ps_z.tile([128, 128], f32, tag="z", name="psum_z")
                for i, (m, kk) in enumerate(PIECES[p]):
                    nc.tensor.matmul(psum_z[:ww, :],
                                     lhsT=aggT_all[:, m, 128 * w:128 * w + ww],
                                     rhs=wx_t[:, kk, :], start=(i == 0), stop=False)
                nc.tensor.matmul(psum_z[:ww, :], lhsT=eaT_t[:, w, :ww],
                                 rhs=wep_t[:, p, :], start=False, stop=True)
                last = p == _NPROD - 1
                z_t = zpool.tile([128, 128], f32 if last else fp16,
                                 tag="z32" if last else "z16", name="z_t")
                nc.vector.tensor_scalar(
                    out=z_t[:ww, :], in0=psum_z[:ww, :],
                    scalar1=recip_all[:ww, w:w + 1], scalar2=None,
                    op0=mybir.AluOpType.mult,
                )
                if last:
                    nc.sync.dma_start(out=out_d[128 * w:128 * w + ww, :],
                                      in_=z_t[:ww, :])
                elif w < WA:
                    nc.sync.dma_start(out=zinA[p][128 * w:128 * (w + 1), :], in_=z_t[:, :])
                else:
                    nc.sync.dma_start(out=zinB[p][128 * (w - WA):128 * (w - WA) + ww, :],
                                      in_=z_t[:ww, :])

            def publish(p, k, psk):
                a, b = CHUNKS[k]
                nc.vector.tensor_copy(out=aggT_all[:, p, a:b], in_=psk[:, :])
                for w in CHUNK_WINS[k]:
                    emit_z(p, w)
                if p < _NCC and p < _NPROD - 1:
                    if k == 0:
                        nc.gpsimd.collective_compute(
                            "AllGather", mybir.AluOpType.bypass,
                            replica_groups=[list(range(N_CORES))],
                            ins=[zinA[p].opt()], outs=[zfullA[p].opt()],
                        )
                    elif k == 2:
                        nc.gpsimd.collective_compute(
                            "AllGather", mybir.AluOpType.bypass,
                            replica_groups=[list(range(N_CORES))],
                            ins=[zinB[p].opt()], outs=[zfullB[p].opt()],
                        )

            for p in range(_NPROD):
                ht = hpool.tile([128, ST, CH], fp16, tag="ht", name="ht")
                if p == 0:
                    # x16 on the sync ring; at blocks stream on scalar ring
                    for i in range(0, ST, 8):
                        nc.sync.dma_start(
                            out=ht[:, i:i + 8, :],
                            in_=x16[:, i * CH:(i + 8) * CH]
                            .rearrange("q (s c) -> q s c", c=CH))
                    for k in range(3):
                        for i in range(0, ST, 8):
                            nc.scalar.dma_start(out=at_t[k][:, i:i + 8, :],
                                                in_=at_d[k][:, i:i + 8, :])
                else:
                    # scalar ring: idle after p0's at loads, so reloads don't
                    # queue behind z stores (sync ring) at product boundaries
                    for i in range(0, STA, 8):
                        nc.scalar.dma_start(
                            out=ht[:, i:i + 8, :],
                            in_=zfullA[p - 1][128 * i:128 * (i + 8), :]
                            .rearrange("(s q) c -> q s c", q=128))
                    for i in range(STA, ST, 8):
                        nc.scalar.dma_start(
                            out=ht[:, i:i + 8, :],
                            in_=zfullB[p - 1][128 * (i - STA):128 * (i - STA + 8), :]
                            .rearrange("(s q) c -> q s c", q=128))

                psk = [ps_agg.tile([128, b - a], f32, tag=f"agg{k}", name=f"agg{k}")
                       for k, (a, b) in enumerate(CHUNKS)]

                def agg_mm(s, k, start, stop):
                    nc.tensor.matmul(psk[k][:, :], lhsT=ht[:, s, :],
                                     rhs=at_t[k][:, s, :], start=start, stop=stop)

                if p == 0:
                    # chunk-major: publish each chunk as early as possible
                    for k in range(3):
                        for s in range(ST):
                            agg_mm(s, k, start=(s == 0), stop=(s == ST - 1))
                        publish(p, k, psk[k])
                else:
                    # A-block: all chunks over A srcs (gathered first)
                    for s in range(STA):
                        for k in range(3):
                            agg_mm(s, k, start=(s == 0), stop=False)
                    # B chunk0, then publish A windows + AllGather-A
                    for s in range(STA, ST):
                        agg_mm(s, 0, start=False, stop=(s == ST - 1))
                    publish(p, 0, psk[0])
                    for s in range(STA, ST):
                        agg_mm(s, 1, start=False, stop=(s == ST - 1))
                    publish(p, 1, psk[1])
                    for s in range(STA, ST):
                        agg_mm(s, 2, start=False, stop=(s == ST - 1))
                    publish(p, 2, psk[2])
    nc.finalize()
    return nc


_CACHE = {}


def _get_program():
    if "nc" not in _CACHE:
        from concourse import mybir, bacc
        import concourse.bass as bass
        import concourse.tile as tile
        _CACHE["nc"] = _build(mybir, bass, tile, bacc)
    return _CACHE["nc"]


def _run(inputs, trace=False, tmpdir=None):
    from concourse.bass_utils import run_bass_kernel_spmd

    x = np.asarray(inputs["x"], np.float32)
    edge_attr = np.asarray(inputs["edge_attr"], np.float32)
    edge_index = np.asarray(inputs["edge_index"])
    Ws = [np.asarray(inputs[f"W{i}"], np.float32) for i in range(4)]
    bs = [np.asarray(inputs[f"b{i}"], np.float32) for i in range(4)]

    # fp8e3 (e3m4) holds integers 0..15 exactly; a >15-fold repeated edge is
    # probabilistically impossible for these inputs, so fail loudly rather
    # than compute wrong sums.
    src_i = np.asarray(edge_index[0], np.int64)
    dst_i = np.asarray(edge_index[1], np.int64)
    _, pair_counts = np.unique(src_i * N_NODES + dst_i, return_counts=True)
    assert int(pair_counts.max()) <= 15, \
        f"edge multiplicity {int(pair_counts.max())} exceeds fp8e3 exact range"
    at, eaT, recip = _prep(edge_index, edge_attr)
    nc = _get_program()

    # x in padded, PERM-tile layout
    xp = np.zeros((NPAD, CH), np.float16)
    for c in range(N_CORES):
        xp[NPP * c:NPP * c + NPC] = x[NPC * c:NPC * (c + 1)].astype(np.float16)
    x16 = xp.reshape(ST, 128, CH)[PERM].transpose(1, 0, 2).reshape(128, ST * CH)

    wx = np.stack([
        Ws[0][:128], Ws[1][:128],
        Ws[2][:128], Ws[2][128:256],
        Ws[3][:128], Ws[3][128:256], Ws[3][256:384],
    ]).astype(np.float16)
    Cs = [128, 128, 256, 384]
    wep = np.stack([
        np.concatenate([Ws[l][Cs[l]:Cs[l] + EDGE_DIM], bs[l][None, :]], axis=0)
        for l in range(4)
    ]).astype(np.float16)

    in_maps = []
    for c in range(N_CORES):
        m = {
            "x16": x16,
            "eaT": eaT[c],
            "recip": recip[c],
            "wx": wx,
            "wep": wep,
        }
        for k in range(3):
            m[f"at{k}"] = at[c][k]
        in_maps.append(m)
    res = run_bass_kernel_spmd(nc, in_maps, core_ids=list(range(N_CORES)),
                               trace=trace, tmpdir=tmpdir)
    out = np.concatenate([res.results[c]["zout"] for c in range(N_CORES)], axis=0)
    return out, res


def kernel(**inputs) -> np.ndarray:
    out, _ = _run(inputs, trace=False)
    return out
